# revision 18
# baseline (speedup 1.0000x reference)
"""Trainium2 Bass kernel for nn_EquivariantProductBasisBlock (MACE product basis).

Per (node b, channel c) the block computes a symmetric cubic polynomial in
x = node_feats[b,c,:] (16-dim); coefficients depend on the node's element via
W[e,k,c] and on the CG tensors U (host-folded, O(weights) prep only):

  F = [x (16) | q = x_i x_j, i<=j (136) | t = q_ij x_m, i<=j<=m (816)]  per (b,c)
  gtd[c,(b,kap)] += F_chunk[:, b-cols].T @ U_chunk  -- per-node transposed G
  Wrep[c, (b,kap)] = WK[kap].T @ attrs.T            -- exact for any node_attrs
  out1[c, (b,ld)]  = sum_kap gtd * Wrep             (GPSIMD mul + add tree)
  out[b] = concat_li(lin_li.T @ out1)/sqrt(C) + sc

The 968 q/t product rows are packed into EIGHT 'units' (u0 = 128 pairs,
u1 = 96 triples + 8 leftover pairs at partition 96, u2..u6 = 128 triples,
u7 = 72 triples + 8 final triples at partition 96). Per unit: two PE
selection matmuls build the operands in PSUM (sa via a 4-deep slot
rotation, sb via 3-deep), ScalarE stages sb to SBUF, the DVE multiplies
(one PSUM read), and per-node bf16 matmuls accumulate transposed-G
directly (no [64,NB] G pass, no per-node transposes, no G stage).
Sharding: data-parallel over nodes, 128 nodes/core on 8 cores, no
collectives.
"""
import math
import os
import numpy as np

N, C, L, E = 1024, 128, 16, 10
NCORES = 8
BLOC = N // NCORES            # nodes per core
NLOC = BLOC * C               # (b,c) columns per core; n = b*C + c
NB = 512                      # column block (one fp32 PSUM bank)
NBLK = NLOC // NB
NNOD = NB // C                # nodes per block
LBLK = (NBLK + 2) // 3        # column blocks per partition lane (X packing)
LANEW = LBLK * NB             # free width per lane

PAIRS = [(i, j) for j in range(L) for i in range(j + 1)]              # 136, j-outer
TRIPLES = [(i, j, m) for j in range(L) for i in range(j + 1) for m in range(j, L)]
NQ, NT = len(PAIRS), len(TRIPLES)                                      # 136, 816
# qA = pairs 0..126 plus (15,15) at row 127. Triples (i,15,15) decompose as
# q(15,15) * x_i, so NO triple needs the 8 leftover pairs (7..14, 15) -- they
# are features only, computed in u1 at partition base 64.
QA_PAIRS = PAIRS[0:127] + [(15, 15)]
QB_PAIRS = PAIRS[127:135]                                  # (7,15)..(14,15)
ROW2 = {p: r for r, p in enumerate(QA_PAIRS)}

def _tdecomp(tr):
    i, j, m = tr
    if j == 15 and m == 15:
        return ROW2[(15, 15)], i                           # q(15,15) * x_i
    return ROW2[(i, j)], m                                 # q(i,j) * x_m

# unit t-slices: u1 gets TRIPLES[0:64] (+ 8 qB pairs at rows 64..71),
# u2..u6 get 128 each, u7 gets the last 112
T_U = [(0, 64)] + [(64 + 128 * k, 128) for k in range(5)] + [(704, 112)]
NUNIT = 8
U_BLK = [16, 128, 72, 128, 128, 128, 128, 128, 112]    # gtd K per feature tile
URON = [128, 72, 128, 128, 128, 128, 128, 112]         # product rows per unit


def _build_consts(inputs, np_mmdt):
    f32 = np.float32
    Us = [{nu: np.asarray(inputs[f"U_{li}_{nu}"], f32) for nu in (1, 2, 3)} for li in range(2)]
    Ws = [{nu: np.asarray(inputs[f"W_{li}_{nu}"], f32) for nu in (1, 2, 3)} for li in range(2)]
    lins = [np.asarray(inputs[f"lin_{li}"], f32) for li in range(2)]

    Ubig = np.zeros((16 + NQ + NT, 64), np.float64)
    for ld in range(4):
        li, dd = (0, 0) if ld == 0 else (1, ld - 1)
        U3, U2, U1 = (np.asarray(Us[li][nu], np.float64) for nu in (3, 2, 1))
        Ubig[0:16, ld * 16 + 15] = U1[dd, :, 0]
        for r, (i, j) in enumerate(PAIRS):
            v = U2[dd, i, j, :] + (U2[dd, j, i, :] if i != j else 0.0)
            Ubig[16 + r, ld * 16 + 11:ld * 16 + 15] = v
        for r, (i, j, m) in enumerate(TRIPLES):
            if i < j < m:
                arr = [(i, j, m), (i, m, j), (j, i, m), (j, m, i), (m, i, j), (m, j, i)]
            elif i == j and j < m:
                arr = [(i, i, m), (i, m, i), (m, i, i)]
            elif i < j and j == m:
                arr = [(i, j, j), (j, i, j), (j, j, i)]
            else:
                arr = [(i, i, i)]
            Ubig[16 + NQ + r, ld * 16:ld * 16 + 11] = sum(U3[dd, a, b, c, :] for (a, b, c) in arr)
    Ubig = Ubig.astype(f32)
    UX = Ubig[0:16]                      # x features
    UQ = Ubig[16:16 + NQ]                # pair features (136, original order)
    UT = Ubig[16 + NQ:]                  # triple features (816)

    # per-unit U blocks, padded to the unit row layouts
    U_all = np.zeros((128, 64 * (NUNIT + 1)), f32)
    U_all[:16, 0:64] = UX
    for r, p in enumerate(QA_PAIRS):                      # u0: qA (remapped)
        U_all[r, 64:128] = UQ[PAIRS.index(p)]
    for u in range(1, NUNIT):
        o, cnt = T_U[u - 1]
        blkc = np.zeros((128, 64), f32)
        blkc[0:cnt] = UT[o:o + cnt]
        if u == 1:
            for c, p in enumerate(QB_PAIRS):              # leftover pairs
                blkc[64 + c] = UQ[PAIRS.index(p)]
        U_all[:, (u + 1) * 64:(u + 2) * 64] = blkc

    # selection matrices (0/1) for the replication matmuls
    XI0 = np.zeros((16, 128), f32); XJ0 = np.zeros((16, 128), f32)
    for r, (i, j) in enumerate(QA_PAIRS):
        XI0[i, r] = 1.0; XJ0[j, r] = 1.0
    SA1B = np.zeros((16, 8), f32)                         # xi of qB pairs
    for c, (i, j) in enumerate(QB_PAIRS):
        SA1B[i, c] = 1.0
    QISM = np.zeros((128, NT), f32)                       # qA-row per triple
    XMSM = np.zeros((16, NT), f32)                        # x-side per triple
    for r, tr in enumerate(TRIPLES):
        qrow, xrow = _tdecomp(tr)
        QISM[qrow, r] = 1.0
        XMSM[xrow, r] = 1.0
    # u1 sb: cols 0..63 x-side of TRIPLES[0:64], 64..71 xj (= x15) of qB
    SB1 = np.zeros((16, 72), f32)
    SB1[:, 0:64] = XMSM[:, 0:64]
    SB1[15, 64:72] = 1.0

    WKp = np.zeros((E, 64, C), f32)
    for ld in range(4):
        li = 0 if ld == 0 else 1
        WKp[:, ld * 16:ld * 16 + 11, :] = Ws[li][3]
        WKp[:, ld * 16 + 11:ld * 16 + 15, :] = Ws[li][2]
        WKp[:, ld * 16 + 15, :] = Ws[li][1][:, 0, :]

    # 3-lane packing: small stationary matrices replicated at partition bases
    # {0,32,64} so lhsT.base == rhs.base for the lane-packed X (SBUF economy).
    def lane3(mat):
        rows = mat.shape[0]
        out = np.zeros((64 + rows, mat.shape[1]), mat.dtype)
        for Lb in range(3):
            out[32 * Lb:32 * Lb + rows] = mat
        return out

    # WKp packed at 3 bases: kappa groups 0..23 | 24..47 | 48..63 (multiples of
    # 4 so each Wrep round's matmuls share one partition base / PE row group)
    WK3 = np.zeros((74, 24 * C), f32)
    for kap in range(64):
        g, off = (0, 0) if kap < 24 else ((1, 24) if kap < 48 else (2, 48))
        WK3[32 * g:32 * g + E, (kap - off) * C:(kap - off + 1) * C] = WKp[:, kap, :]

    isc = f32(1.0 / math.sqrt(C))
    return {
        "U_all": U_all.astype(np_mmdt), "UX3": lane3(UX).astype(np_mmdt),
        "XI0": lane3(XI0).astype(np_mmdt), "XJ0": lane3(XJ0).astype(np_mmdt),
        "SA1B": lane3(SA1B).astype(np_mmdt), "SB1": lane3(SB1).astype(np_mmdt),
        "QISM": QISM.astype(np_mmdt), "XMSM": lane3(XMSM).astype(np_mmdt),
        "WK3": WK3,
        "lin0": np.ascontiguousarray(lins[0] * isc),
        "lin1": np.ascontiguousarray(lins[1] * isc),
    }


def build_program():
    import concourse.bass as bass
    import concourse.bacc as bacc
    import concourse.mybir as mybir
    import concourse.tile as tile
    from concourse.masks import make_identity
    from contextlib import ExitStack

    dt = mybir.dt
    F32 = dt.float32
    BF16 = dt.bfloat16
    SDT = BF16

    nc = bacc.Bacc(None, target_bir_lowering=False)
    X_Tm = nc.dram_tensor("X_Tm", [80, LANEW], BF16, kind="ExternalInput")
    attrsT = nc.dram_tensor("attrsT", [E, BLOC], F32, kind="ExternalInput")
    sc_d = nc.dram_tensor("sc", [BLOC, 512], F32, kind="ExternalInput")
    U_all = nc.dram_tensor("U_all", [128, 64 * (NUNIT + 1)], BF16, kind="ExternalInput")
    UX3 = nc.dram_tensor("UX3", [80, 64], BF16, kind="ExternalInput")
    XI0 = nc.dram_tensor("XI0", [80, 128], BF16, kind="ExternalInput")
    XJ0 = nc.dram_tensor("XJ0", [80, 128], BF16, kind="ExternalInput")
    SA1B = nc.dram_tensor("SA1B", [80, 8], BF16, kind="ExternalInput")
    SB1 = nc.dram_tensor("SB1", [80, 72], BF16, kind="ExternalInput")
    QISM = nc.dram_tensor("QISM", [128, NT], BF16, kind="ExternalInput")
    XMSM = nc.dram_tensor("XMSM", [80, NT], BF16, kind="ExternalInput")
    WK3 = nc.dram_tensor("WK3", [74, 24 * C], F32, kind="ExternalInput")
    lin0 = nc.dram_tensor("lin0", [C, C], F32, kind="ExternalInput")
    lin1 = nc.dram_tensor("lin1", [C, C], F32, kind="ExternalInput")
    OUT = nc.dram_tensor("OUT", [BLOC, 512], F32, kind="ExternalOutput")

    with tile.TileContext(nc) as tc, ExitStack() as ctx:
        cpool = ctx.enter_context(tc.tile_pool(name="consts", bufs=1))
        fpool = ctx.enter_context(tc.tile_pool(name="feats", bufs=2))
        spool = ctx.enter_context(tc.tile_pool(name="stage", bufs=4))
        # PSUM budget (8 banks): sa slots x4 (DVE-read side; slot-reuse cycle
        # mm->stage->mul->mm spans 4 units), sb slots x3 (staged side), gtd
        # accumulator (1). Prologue wps and lin-tail tiles borrow sa slots.
        psa = ctx.enter_context(tc.tile_pool(name="ps_sa", bufs=4, space="PSUM"))
        psb = ctx.enter_context(tc.tile_pool(name="ps_sb", bufs=3, space="PSUM"))
        pp_g = ctx.enter_context(tc.tile_pool(name="ps_g", bufs=1, space="PSUM"))

        # PE-consumed tiles are laundered through one copy each, so any
        # matmul's operand producers collapse onto a single engine.
        def launder(shape, dtp, tag, src):
            raw = cpool.tile(shape, src.dtype, tag=tag + "_r")
            nc.sync.dma_start(raw[:], src[:])
            t = cpool.tile(shape, dtp, tag=tag)
            nc.vector.tensor_copy(t[:], raw[:])
            return t

        xsm = launder([80, LANEW], SDT, "xTm", X_Tm)
        ua = launder([128, 64 * (NUNIT + 1)], SDT, "uall", U_all)
        ux3 = launder([80, 64], SDT, "ux3", UX3)
        xi0 = launder([80, 128], SDT, "xi0", XI0)
        xj0 = launder([80, 128], SDT, "xj0", XJ0)
        sa1b = launder([80, 8], SDT, "sa1b", SA1B)
        sb1 = launder([80, 72], SDT, "sb1", SB1)
        qism = launder([128, NT], SDT, "qism", QISM)
        xmsm = launder([80, NT], SDT, "xmsm", XMSM)
        wk3 = launder([74, 24 * C], BF16, "wk3", WK3)
        l0 = launder([C, C], BF16, "lin0", lin0)
        l1 = launder([C, C], BF16, "lin1", lin1)
        # attrs replicated at the 3 bases to pair with WK3 lhsT slices (bf16:
        # one-hot indicator rows are exact in bf16, W rounds at ~0.4%)
        ats_raw = cpool.tile([74, BLOC], F32, tag="attrs_r")
        ats = cpool.tile([74, BLOC], BF16, tag="attrs")
        for Lb in range(3):
            nc.sync.dma_start(ats_raw[32 * Lb:32 * Lb + E], attrsT[:])
            nc.vector.tensor_copy(ats[32 * Lb:32 * Lb + E], ats_raw[32 * Lb:32 * Lb + E])
        sct = cpool.tile([BLOC, 512], F32, tag="sc"); nc.sync.dma_start(sct[:], sc_d[:])
        ident_raw = cpool.tile([128, 128], F32, tag="ident_r")
        make_identity(nc, ident_raw[:])
        ident = cpool.tile([128, 128], F32, tag="ident")
        nc.vector.tensor_copy(ident[:], ident_raw[:])

        # Wrep' [c, (b, kap)] b-major so each block's p-mul slice is
        # contiguous: 16 rounds x 4 kappa, K=10 bf16 matmuls; the PSUM->SBUF
        # copies alternate DVE/ScalarE (prologue, before the block stream).
        wrep = cpool.tile([C, BLOC * 64], F32, tag="wrep")
        wrv = wrep[:].rearrange("c (b k) -> c b k", k=64)
        for rnd in range(16):
            wps = psa.tile([128, NB], F32, tag="sa")
            for kk in range(4):
                kap = rnd * 4 + kk
                g3, off = (0, 0) if kap < 24 else ((1, 24) if kap < 48 else (2, 48))
                nc.tensor.matmul(
                    wps[:, kk * BLOC:(kk + 1) * BLOC],
                    wk3[32 * g3:32 * g3 + E, (kap - off) * C:(kap - off + 1) * C],
                    ats[32 * g3:32 * g3 + E], start=True, stop=True)
            if rnd % 2 == 0:
                nc.vector.tensor_copy(wrv[:, :, rnd * 4:(rnd + 1) * 4],
                                      wps[:].rearrange("c (k b) -> c b k", k=4))
            else:
                nc.scalar.copy(wrv[:, :, rnd * 4:(rnd + 1) * 4],
                               wps[:].rearrange("c (k b) -> c b k", k=4))

        out1 = cpool.tile([C, BLOC * 4], F32, tag="out1")  # [c, (b, ld)]

        deferred = []          # p-mul + reduce-tree emission deferred 1 block

        def emit_tail(blk, gtd):
            b0 = blk * NNOD
            gtd_sb = fpool.tile([C, NNOD * 64], F32, tag="gtd_sb")
            nc.scalar.copy(gtd_sb[:], gtd[:])
            p_sb = fpool.tile([C, NNOD * 64], F32, tag="p_sb")
            wr_v = wrv[:, b0:b0 + NNOD, :]
            nc.gpsimd.tensor_mul(p_sb[:].rearrange("c (b k) -> c b k", b=NNOD),
                                 gtd_sb[:].rearrange("c (b k) -> c b k", b=NNOD),
                                 wr_v)
            # kappa-sum (within each ld's 16 paths) as a log2 add tree on the
            # GPSIMD engine, freeing DVE of the TensorReduce
            pv = p_sb[:].rearrange("c (q k) -> c q k", k=16)      # q = (b, l)
            r8 = fpool.tile([C, NNOD * 4 * 8], F32, tag="r8")
            nc.gpsimd.tensor_add(r8[:].rearrange("c (q k) -> c q k", k=8),
                                 pv[:, :, 0:8], pv[:, :, 8:16])
            rv8 = r8[:].rearrange("c (q k) -> c q k", k=8)
            r4 = fpool.tile([C, NNOD * 4 * 4], F32, tag="r4")
            nc.gpsimd.tensor_add(r4[:].rearrange("c (q k) -> c q k", k=4),
                                 rv8[:, :, 0:4], rv8[:, :, 4:8])
            rv4 = r4[:].rearrange("c (q k) -> c q k", k=4)
            r2 = fpool.tile([C, NNOD * 4 * 2], F32, tag="r2")
            nc.gpsimd.tensor_add(r2[:].rearrange("c (q k) -> c q k", k=2),
                                 rv4[:, :, 0:2], rv4[:, :, 2:4])
            rv2 = r2[:].rearrange("c (q k) -> c q k", k=2)
            nc.gpsimd.tensor_add(
                out1[:, b0 * 4:(b0 + NNOD) * 4].rearrange("c (q k) -> c q k", k=1),
                rv2[:, :, 0:1], rv2[:, :, 1:2])

        # Replication matmuls for unit u of a block. u1 reads qA (u0's
        # product); its 8 leftover-pair rows ride at partition base 64.
        def emit_repl(u, p0, xsm_b, prods):
            sa = psa.tile([128, NB], F32, tag="sa")
            sb = psb.tile([128, NB], F32, tag="sb")
            if u == 0:
                nc.tensor.matmul(sa[:128], xi0[p0:p0 + 16], xsm_b,
                                 start=True, stop=True)
                nc.tensor.matmul(sb[:128], xj0[p0:p0 + 16], xsm_b,
                                 start=True, stop=True)
            elif u == 1:
                nc.tensor.matmul(sa[:64], qism[:, 0:64], prods[0][:128],
                                 start=True, stop=True)
                nc.tensor.matmul(sa[64:72], sa1b[p0:p0 + 16], xsm_b,
                                 start=True, stop=True)
                nc.tensor.matmul(sb[:72], sb1[p0:p0 + 16], xsm_b,
                                 start=True, stop=True)
            else:
                o, cnt = T_U[u - 1]
                nc.tensor.matmul(sa[:cnt], qism[:, o:o + cnt], prods[0][:128],
                                 start=True, stop=True)
                nc.tensor.matmul(sb[:cnt], xmsm[p0:p0 + 16, o:o + cnt], xsm_b,
                                 start=True, stop=True)
            return sa, sb


        repl = None
        pending = None         # (blk, prods, xsm_b, p0) awaiting gtd burst

        def emit_gtd(pblk, pprods, pxsm_b, pp0):
            # gtd burst: G accumulated transposed, [c, (node, kap)]. One
            # sequential accumulation group per node region (an open matmul
            # group owns its whole PSUM bank, so groups must not interleave).
            # Emitted one block late so every product is long complete and
            # the PE never stalls mid-burst.
            gtd = pp_g.tile([C, NNOD * 64], F32, tag="g")
            for bb in range(NNOD):
                reg = gtd[:, bb * 64:(bb + 1) * 64]
                cs = slice(bb * C, (bb + 1) * C)
                nc.tensor.matmul(reg, pxsm_b[:, cs], ux3[pp0:pp0 + 16, 0:64],
                                 start=True, stop=False)
                for u in range(NUNIT):
                    nc.tensor.matmul(reg, pprods[u][:U_BLK[u + 1], cs],
                                     ua[:U_BLK[u + 1], (u + 1) * 64:(u + 2) * 64],
                                     start=False, stop=u == NUNIT - 1)
            deferred.append((pblk, gtd))

        for blk in range(NBLK):
            Lb = blk // LBLK
            p0 = 32 * Lb
            csl = slice((blk % LBLK) * NB, (blk % LBLK + 1) * NB)
            xsm_b = xsm[p0:p0 + 16, csl]

            prods = []
            if repl is None:
                repl = emit_repl(0, p0, xsm_b, prods)

            # ---- product units ----
            for u in range(NUNIT):
                sa, sb = repl
                rows = URON[u]
                st = spool.tile([128, NB], F32, tag="st")
                nc.scalar.copy(st[:rows], sb[:rows])
                pr = fpool.tile([128, NB], SDT, tag=f"u{u}")
                # u1's replication reads the u0 product, so it can only be
                # emitted after u0's multiply; all other units pre-emit
                # their successor's replication to keep the PE ahead
                if u > 0 and u + 1 < NUNIT:
                    repl = emit_repl(u + 1, p0, xsm_b, prods)
                elif u == NUNIT - 1 and blk + 1 < NBLK:
                    Lb1 = (blk + 1) // LBLK
                    q0 = 32 * Lb1
                    csl1 = slice(((blk + 1) % LBLK) * NB,
                                 ((blk + 1) % LBLK + 1) * NB)
                    repl = emit_repl(0, q0, xsm[q0:q0 + 16, csl1], [])
                nc.vector.tensor_mul(pr[:rows], sa[:rows], st[:rows])
                prods.append(pr)
                if u == 0:
                    repl = emit_repl(1, p0, xsm_b, prods)
                if u == 3 and deferred:
                    emit_tail(*deferred.pop(0))

            # previous block's gtd burst goes here: it fills the PE's natural
            # stall window while it waits for the next block's u0 product
            if pending is not None:
                emit_gtd(*pending)
            pending = (blk, prods, xsm_b, p0)

        if deferred:
            emit_tail(*deferred.pop(0))
        emit_gtd(*pending)
        while deferred:
            emit_tail(*deferred.pop(0))

        _tail(nc, tc, fpool, psa, out1, l0, l1, sct, ident, OUT, F32)
    nc.compile()
    return nc


def _tail(nc, tc, fpool, psa, out1, l0, l1, sct, ident, OUT, F32):
        import concourse.mybir as mybir
        # lin matmuls run in bf16 (l0/l1 laundered to bf16; out1 recast)
        o1b = fpool.tile([C, BLOC * 4], mybir.dt.bfloat16, tag="o1b")
        nc.vector.tensor_copy(o1b[:], out1[:])
        o1v = o1b[:].rearrange("c (b l) -> c b l", l=4)
        lo_ps = psa.tile([128, NB], F32, tag="sa")
        nc.tensor.matmul(lo_ps[:C, :BLOC], l0[:], o1v[:, :, 0], start=True, stop=True)
        l1_ps = psa.tile([128, NB], F32, tag="sa")
        nc.tensor.matmul(l1_ps[:C, :BLOC * 3].rearrange("f (b d) -> f b d", d=3),
                         l1[:], o1v[:, :, 1:4], start=True, stop=True)
        lo_sb = fpool.tile([C, BLOC], F32, tag="lo_sb")
        nc.vector.tensor_copy(lo_sb[:], lo_ps[:C, :BLOC])
        l1_sb = fpool.tile([C, BLOC * 3], F32, tag="l1_sb")
        nc.vector.tensor_copy(l1_sb[:], l1_ps[:C, :BLOC * 3])
        outt = fpool.tile([BLOC, 512], F32, tag="outt")
        tps = psa.tile([128, NB], F32, tag="sa")
        nc.tensor.transpose(tps[:BLOC, :C], lo_sb[:], ident[:])
        nc.vector.tensor_add(outt[:, 0:128], tps[:BLOC, :C], sct[:, 0:128])
        l1v = l1_sb[:].rearrange("f (b d) -> f b d", d=3)
        o_v = outt[:, 128:].rearrange("b (f d) -> b f d", d=3)
        s_v = sct[:, 128:].rearrange("b (f d) -> b f d", d=3)
        for ddi in range(3):
            tpd = psa.tile([128, NB], F32, tag="sa")
            nc.tensor.transpose(tpd[:BLOC, :C], l1v[:, :, ddi], ident[:])
            nc.vector.tensor_add(o_v[:, :, ddi], tpd[:BLOC, :C], s_v[:, :, ddi])
        nc.sync.dma_start(OUT[:], outt[:])


_PROG = {}


def kernel(**inputs):
    import concourse.bass_utils as bass_utils
    import ml_dtypes
    np_mmdt = ml_dtypes.bfloat16
    consts = _build_consts(inputs, np_mmdt)

    nf = np.asarray(inputs["node_feats"], np.float32)
    attrs = np.asarray(inputs["node_attrs"], np.float32)
    sc = np.asarray(inputs["sc"], np.float32)

    if "prog" not in _PROG:
        _PROG["prog"] = build_program()
    nc = _PROG["prog"]

    in_maps = []
    for r in range(NCORES):
        b0 = r * BLOC
        xt = nf[b0:b0 + BLOC].transpose(2, 0, 1).reshape(16, NLOC)
        # 3-lane pack: lane Lb at partition base 32*Lb holds column blocks
        # [Lb*LBLK, (Lb+1)*LBLK)
        x3 = np.zeros((80, LANEW), np.float32)
        for blk in range(NBLK):
            Lb, cb = blk // LBLK, blk % LBLK
            x3[32 * Lb:32 * Lb + 16, cb * NB:(cb + 1) * NB] = xt[:, blk * NB:(blk + 1) * NB]
        m = {"X_Tm": x3.astype(np_mmdt),
             "attrsT": np.ascontiguousarray(attrs[b0:b0 + BLOC].T),
             "sc": np.ascontiguousarray(sc[b0:b0 + BLOC])}
        m.update(consts)
        in_maps.append(m)

    res = bass_utils.run_bass_kernel_spmd(
        nc, in_maps, list(range(NCORES)),
        trace=os.environ.get("KTRACE", "0") == "1")
    global LAST_EXEC_NS
    LAST_EXEC_NS = getattr(res, "exec_time_ns", None)
    outs = [np.asarray(res.results[r]["OUT"]) for r in range(NCORES)]
    return np.concatenate(outs, axis=0).astype(np.float32)


LAST_EXEC_NS = None


# revision 19
# speedup vs baseline: 1.0295x; 1.0295x over previous
"""Trainium2 Bass kernel for nn_EquivariantProductBasisBlock (MACE product basis).

Per (node b, channel c) the block computes a symmetric cubic polynomial in
x = node_feats[b,c,:] (16-dim); coefficients depend on the node's element via
W[e,k,c] and on the CG tensors U (host-folded, O(weights) prep only):

  F = [x (16) | q = x_i x_j, i<=j (136) | t = q_ij x_m, i<=j<=m (816)]  per (b,c)
  gtd[c,(b,kap)] += F_chunk[:, b-cols].T @ U_chunk  -- per-node transposed G
  Wrep[c, (b,kap)] = WK[kap].T @ attrs.T            -- exact for any node_attrs
  out1[c, (b,ld)]  = sum_kap gtd * Wrep             (GPSIMD mul + add tree)
  out[b] = concat_li(lin_li.T @ out1)/sqrt(C) + sc

The 968 q/t product rows are packed into EIGHT 'units' (u0 = 128 pairs,
u1 = 96 triples + 8 leftover pairs at partition 96, u2..u6 = 128 triples,
u7 = 72 triples + 8 final triples at partition 96). Per unit: two PE
selection matmuls build the operands in PSUM (sa via a 4-deep slot
rotation, sb via 3-deep), ScalarE stages sb to SBUF, the DVE multiplies
(one PSUM read), and per-node bf16 matmuls accumulate transposed-G
directly (no [64,NB] G pass, no per-node transposes, no G stage).
Sharding: data-parallel over nodes, 128 nodes/core on 8 cores, no
collectives.
"""
import math
import os
import numpy as np

N, C, L, E = 1024, 128, 16, 10
NCORES = 8
BLOC = N // NCORES            # nodes per core
NLOC = BLOC * C               # (b,c) columns per core; n = b*C + c
NB = 512                      # column block (one fp32 PSUM bank)
NBLK = NLOC // NB
NNOD = NB // C                # nodes per block
LBLK = (NBLK + 2) // 3        # column blocks per partition lane (X packing)
LANEW = LBLK * NB             # free width per lane

PAIRS = [(i, j) for j in range(L) for i in range(j + 1)]              # 136, j-outer
TRIPLES = [(i, j, m) for j in range(L) for i in range(j + 1) for m in range(j, L)]
NQ, NT = len(PAIRS), len(TRIPLES)                                      # 136, 816
# qA = pairs 0..126 plus (15,15) at row 127. Triples (i,15,15) decompose as
# q(15,15) * x_i, so NO triple needs the 8 leftover pairs (7..14, 15) -- they
# are features only, computed in u1 at partition base 64.
QA_PAIRS = PAIRS[0:127] + [(15, 15)]
QB_PAIRS = PAIRS[127:135]                                  # (7,15)..(14,15)
ROW2 = {p: r for r, p in enumerate(QA_PAIRS)}

def _tdecomp(tr):
    i, j, m = tr
    if j == 15 and m == 15:
        return ROW2[(15, 15)], i                           # q(15,15) * x_i
    return ROW2[(i, j)], m                                 # q(i,j) * x_m

# unit t-slices: u1 gets TRIPLES[0:64] (+ 8 qB pairs at rows 64..71),
# u2..u6 get 128 each, u7 gets the last 112
T_U = [(0, 64)] + [(64 + 128 * k, 128) for k in range(5)] + [(704, 112)]
NUNIT = 8
U_BLK = [16, 128, 72, 128, 128, 128, 128, 128, 112]    # gtd K per feature tile
URON = [128, 72, 128, 128, 128, 128, 128, 112]         # product rows per unit


def _build_consts(inputs, np_mmdt):
    f32 = np.float32
    Us = [{nu: np.asarray(inputs[f"U_{li}_{nu}"], f32) for nu in (1, 2, 3)} for li in range(2)]
    Ws = [{nu: np.asarray(inputs[f"W_{li}_{nu}"], f32) for nu in (1, 2, 3)} for li in range(2)]
    lins = [np.asarray(inputs[f"lin_{li}"], f32) for li in range(2)]

    Ubig = np.zeros((16 + NQ + NT, 64), np.float64)
    for ld in range(4):
        li, dd = (0, 0) if ld == 0 else (1, ld - 1)
        U3, U2, U1 = (np.asarray(Us[li][nu], np.float64) for nu in (3, 2, 1))
        Ubig[0:16, ld * 16 + 15] = U1[dd, :, 0]
        for r, (i, j) in enumerate(PAIRS):
            v = U2[dd, i, j, :] + (U2[dd, j, i, :] if i != j else 0.0)
            Ubig[16 + r, ld * 16 + 11:ld * 16 + 15] = v
        for r, (i, j, m) in enumerate(TRIPLES):
            if i < j < m:
                arr = [(i, j, m), (i, m, j), (j, i, m), (j, m, i), (m, i, j), (m, j, i)]
            elif i == j and j < m:
                arr = [(i, i, m), (i, m, i), (m, i, i)]
            elif i < j and j == m:
                arr = [(i, j, j), (j, i, j), (j, j, i)]
            else:
                arr = [(i, i, i)]
            Ubig[16 + NQ + r, ld * 16:ld * 16 + 11] = sum(U3[dd, a, b, c, :] for (a, b, c) in arr)
    Ubig = Ubig.astype(f32)
    UX = Ubig[0:16]                      # x features
    UQ = Ubig[16:16 + NQ]                # pair features (136, original order)
    UT = Ubig[16 + NQ:]                  # triple features (816)

    # per-unit U blocks, padded to the unit row layouts
    U_all = np.zeros((128, 64 * (NUNIT + 1)), f32)
    U_all[:16, 0:64] = UX
    for r, p in enumerate(QA_PAIRS):                      # u0: qA (remapped)
        U_all[r, 64:128] = UQ[PAIRS.index(p)]
    for u in range(1, NUNIT):
        o, cnt = T_U[u - 1]
        blkc = np.zeros((128, 64), f32)
        blkc[0:cnt] = UT[o:o + cnt]
        if u == 1:
            for c, p in enumerate(QB_PAIRS):              # leftover pairs
                blkc[64 + c] = UQ[PAIRS.index(p)]
        U_all[:, (u + 1) * 64:(u + 2) * 64] = blkc

    # selection matrices (0/1) for the replication matmuls
    XI0 = np.zeros((16, 128), f32); XJ0 = np.zeros((16, 128), f32)
    for r, (i, j) in enumerate(QA_PAIRS):
        XI0[i, r] = 1.0; XJ0[j, r] = 1.0
    SA1B = np.zeros((16, 8), f32)                         # xi of qB pairs
    for c, (i, j) in enumerate(QB_PAIRS):
        SA1B[i, c] = 1.0
    QISM = np.zeros((128, NT), f32)                       # qA-row per triple
    XMSM = np.zeros((16, NT), f32)                        # x-side per triple
    for r, tr in enumerate(TRIPLES):
        qrow, xrow = _tdecomp(tr)
        QISM[qrow, r] = 1.0
        XMSM[xrow, r] = 1.0
    # u1 sb: cols 0..63 x-side of TRIPLES[0:64], 64..71 xj (= x15) of qB
    SB1 = np.zeros((16, 72), f32)
    SB1[:, 0:64] = XMSM[:, 0:64]
    SB1[15, 64:72] = 1.0

    WKp = np.zeros((E, 64, C), f32)
    for ld in range(4):
        li = 0 if ld == 0 else 1
        WKp[:, ld * 16:ld * 16 + 11, :] = Ws[li][3]
        WKp[:, ld * 16 + 11:ld * 16 + 15, :] = Ws[li][2]
        WKp[:, ld * 16 + 15, :] = Ws[li][1][:, 0, :]

    # 3-lane packing: small stationary matrices replicated at partition bases
    # {0,32,64} so lhsT.base == rhs.base for the lane-packed X (SBUF economy).
    def lane3(mat):
        rows = mat.shape[0]
        out = np.zeros((64 + rows, mat.shape[1]), mat.dtype)
        for Lb in range(3):
            out[32 * Lb:32 * Lb + rows] = mat
        return out

    # WKp packed at 3 bases: kappa groups 0..23 | 24..47 | 48..63 (multiples of
    # 4 so each Wrep round's matmuls share one partition base / PE row group)
    WK3 = np.zeros((74, 24 * C), f32)
    for kap in range(64):
        g, off = (0, 0) if kap < 24 else ((1, 24) if kap < 48 else (2, 48))
        WK3[32 * g:32 * g + E, (kap - off) * C:(kap - off + 1) * C] = WKp[:, kap, :]

    isc = f32(1.0 / math.sqrt(C))
    return {
        "U_all": U_all.astype(np_mmdt), "UX3": lane3(UX).astype(np_mmdt),
        "XI0": lane3(XI0).astype(np_mmdt), "XJ0": lane3(XJ0).astype(np_mmdt),
        "SA1B": lane3(SA1B).astype(np_mmdt), "SB1": lane3(SB1).astype(np_mmdt),
        "QISM": QISM.astype(np_mmdt), "XMSM": lane3(XMSM).astype(np_mmdt),
        "WK3": WK3,
        "lin0": np.ascontiguousarray(lins[0] * isc),
        "lin1": np.ascontiguousarray(lins[1] * isc),
    }


def build_program():
    import concourse.bass as bass
    import concourse.bacc as bacc
    import concourse.mybir as mybir
    import concourse.tile as tile
    from concourse.masks import make_identity
    from contextlib import ExitStack

    dt = mybir.dt
    F32 = dt.float32
    BF16 = dt.bfloat16
    SDT = BF16

    nc = bacc.Bacc(None, target_bir_lowering=False)
    X_Tm = nc.dram_tensor("X_Tm", [80, LANEW], BF16, kind="ExternalInput")
    attrsT = nc.dram_tensor("attrsT", [E, BLOC], F32, kind="ExternalInput")
    sc_d = nc.dram_tensor("sc", [BLOC, 512], F32, kind="ExternalInput")
    U_all = nc.dram_tensor("U_all", [128, 64 * (NUNIT + 1)], BF16, kind="ExternalInput")
    UX3 = nc.dram_tensor("UX3", [80, 64], BF16, kind="ExternalInput")
    XI0 = nc.dram_tensor("XI0", [80, 128], BF16, kind="ExternalInput")
    XJ0 = nc.dram_tensor("XJ0", [80, 128], BF16, kind="ExternalInput")
    SA1B = nc.dram_tensor("SA1B", [80, 8], BF16, kind="ExternalInput")
    SB1 = nc.dram_tensor("SB1", [80, 72], BF16, kind="ExternalInput")
    QISM = nc.dram_tensor("QISM", [128, NT], BF16, kind="ExternalInput")
    XMSM = nc.dram_tensor("XMSM", [80, NT], BF16, kind="ExternalInput")
    WK3 = nc.dram_tensor("WK3", [74, 24 * C], F32, kind="ExternalInput")
    lin0 = nc.dram_tensor("lin0", [C, C], F32, kind="ExternalInput")
    lin1 = nc.dram_tensor("lin1", [C, C], F32, kind="ExternalInput")
    OUT = nc.dram_tensor("OUT", [BLOC, 512], F32, kind="ExternalOutput")

    with tile.TileContext(nc) as tc, ExitStack() as ctx:
        cpool = ctx.enter_context(tc.tile_pool(name="consts", bufs=1))
        fpool = ctx.enter_context(tc.tile_pool(name="feats", bufs=2))
        spool = ctx.enter_context(tc.tile_pool(name="stage", bufs=4))
        # PSUM budget (8 banks): sa slots x4 (DVE-read side; slot-reuse cycle
        # mm->stage->mul->mm spans 4 units), sb slots x3 (staged side), gtd
        # accumulator (1). Prologue wps and lin-tail tiles borrow sa slots.
        psa = ctx.enter_context(tc.tile_pool(name="ps_sa", bufs=4, space="PSUM"))
        psb = ctx.enter_context(tc.tile_pool(name="ps_sb", bufs=3, space="PSUM"))
        pp_g = ctx.enter_context(tc.tile_pool(name="ps_g", bufs=1, space="PSUM"))

        # PE-consumed tiles are laundered through one copy each, so any
        # matmul's operand producers collapse onto a single engine.
        def launder(shape, dtp, tag, src):
            raw = cpool.tile(shape, src.dtype, tag=tag + "_r")
            nc.sync.dma_start(raw[:], src[:])
            t = cpool.tile(shape, dtp, tag=tag)
            nc.vector.tensor_copy(t[:], raw[:])
            return t

        xsm = launder([80, LANEW], SDT, "xTm", X_Tm)
        ua = launder([128, 64 * (NUNIT + 1)], SDT, "uall", U_all)
        ux3 = launder([80, 64], SDT, "ux3", UX3)
        xi0 = launder([80, 128], SDT, "xi0", XI0)
        xj0 = launder([80, 128], SDT, "xj0", XJ0)
        sa1b = launder([80, 8], SDT, "sa1b", SA1B)
        sb1 = launder([80, 72], SDT, "sb1", SB1)
        qism = launder([128, NT], SDT, "qism", QISM)
        xmsm = launder([80, NT], SDT, "xmsm", XMSM)
        wk3 = launder([74, 24 * C], BF16, "wk3", WK3)
        l0 = launder([C, C], BF16, "lin0", lin0)
        l1 = launder([C, C], BF16, "lin1", lin1)
        # attrs replicated at the 3 bases to pair with WK3 lhsT slices (bf16:
        # one-hot indicator rows are exact in bf16, W rounds at ~0.4%)
        ats_raw = cpool.tile([74, BLOC], F32, tag="attrs_r")
        ats = cpool.tile([74, BLOC], BF16, tag="attrs")
        for Lb in range(3):
            nc.sync.dma_start(ats_raw[32 * Lb:32 * Lb + E], attrsT[:])
            nc.vector.tensor_copy(ats[32 * Lb:32 * Lb + E], ats_raw[32 * Lb:32 * Lb + E])
        sct = cpool.tile([BLOC, 512], F32, tag="sc"); nc.sync.dma_start(sct[:], sc_d[:])
        ident_raw = cpool.tile([128, 128], F32, tag="ident_r")
        make_identity(nc, ident_raw[:])
        ident = cpool.tile([128, 128], F32, tag="ident")
        nc.vector.tensor_copy(ident[:], ident_raw[:])

        # Wrep' [c, (b, kap)] b-major so each block's p-mul slice is
        # contiguous: 16 rounds x 4 kappa, K=10 bf16 matmuls; the PSUM->SBUF
        # copies alternate DVE/ScalarE (prologue, before the block stream).
        wrep = cpool.tile([C, BLOC * 64], F32, tag="wrep")
        wrv = wrep[:].rearrange("c (b k) -> c b k", k=64)
        for rnd in range(16):
            wps = psa.tile([128, NB], F32, tag="sa")
            for kk in range(4):
                kap = rnd * 4 + kk
                g3, off = (0, 0) if kap < 24 else ((1, 24) if kap < 48 else (2, 48))
                nc.tensor.matmul(
                    wps[:, kk * BLOC:(kk + 1) * BLOC],
                    wk3[32 * g3:32 * g3 + E, (kap - off) * C:(kap - off + 1) * C],
                    ats[32 * g3:32 * g3 + E], start=True, stop=True)
            if rnd % 2 == 0:
                nc.vector.tensor_copy(wrv[:, :, rnd * 4:(rnd + 1) * 4],
                                      wps[:].rearrange("c (k b) -> c b k", k=4))
            else:
                nc.scalar.copy(wrv[:, :, rnd * 4:(rnd + 1) * 4],
                               wps[:].rearrange("c (k b) -> c b k", k=4))

        out1 = cpool.tile([C, BLOC * 4], F32, tag="out1")  # [c, (b, ld)]

        deferred = []          # p-mul + reduce-tree emission deferred 1 block

        def emit_tail(blk, gtd):
            b0 = blk * NNOD
            gtd_sb = fpool.tile([C, NNOD * 64], F32, tag="gtd_sb")
            nc.scalar.copy(gtd_sb[:], gtd[:])
            p_sb = fpool.tile([C, NNOD * 64], F32, tag="p_sb")
            wr_v = wrv[:, b0:b0 + NNOD, :]
            nc.gpsimd.tensor_mul(p_sb[:].rearrange("c (b k) -> c b k", b=NNOD),
                                 gtd_sb[:].rearrange("c (b k) -> c b k", b=NNOD),
                                 wr_v)
            # kappa-sum (within each ld's 16 paths) as a log2 add tree on the
            # GPSIMD engine, freeing DVE of the TensorReduce
            pv = p_sb[:].rearrange("c (q k) -> c q k", k=16)      # q = (b, l)
            r8 = fpool.tile([C, NNOD * 4 * 8], F32, tag="r8")
            nc.gpsimd.tensor_add(r8[:].rearrange("c (q k) -> c q k", k=8),
                                 pv[:, :, 0:8], pv[:, :, 8:16])
            rv8 = r8[:].rearrange("c (q k) -> c q k", k=8)
            r4 = fpool.tile([C, NNOD * 4 * 4], F32, tag="r4")
            nc.gpsimd.tensor_add(r4[:].rearrange("c (q k) -> c q k", k=4),
                                 rv8[:, :, 0:4], rv8[:, :, 4:8])
            rv4 = r4[:].rearrange("c (q k) -> c q k", k=4)
            r2 = fpool.tile([C, NNOD * 4 * 2], F32, tag="r2")
            nc.gpsimd.tensor_add(r2[:].rearrange("c (q k) -> c q k", k=2),
                                 rv4[:, :, 0:2], rv4[:, :, 2:4])
            rv2 = r2[:].rearrange("c (q k) -> c q k", k=2)
            nc.gpsimd.tensor_add(
                out1[:, b0 * 4:(b0 + NNOD) * 4].rearrange("c (q k) -> c q k", k=1),
                rv2[:, :, 0:1], rv2[:, :, 1:2])

        # Replication matmuls for unit u of a block. u1 reads qA (u0's
        # product); its 8 leftover-pair rows ride at partition base 64.
        def emit_repl(u, p0, xsm_b, prods):
            sa = psa.tile([128, NB], F32, tag="sa")
            sb = psb.tile([128, NB], F32, tag="sb")
            if u == 0:
                nc.tensor.matmul(sa[:128], xi0[p0:p0 + 16], xsm_b,
                                 start=True, stop=True)
                nc.tensor.matmul(sb[:128], xj0[p0:p0 + 16], xsm_b,
                                 start=True, stop=True)
            elif u == 1:
                nc.tensor.matmul(sa[:64], qism[:, 0:64], prods[0][:128],
                                 start=True, stop=True)
                nc.tensor.matmul(sa[64:72], sa1b[p0:p0 + 16], xsm_b,
                                 start=True, stop=True)
                nc.tensor.matmul(sb[:72], sb1[p0:p0 + 16], xsm_b,
                                 start=True, stop=True)
            else:
                o, cnt = T_U[u - 1]
                nc.tensor.matmul(sa[:cnt], qism[:, o:o + cnt], prods[0][:128],
                                 start=True, stop=True)
                nc.tensor.matmul(sb[:cnt], xmsm[p0:p0 + 16, o:o + cnt], xsm_b,
                                 start=True, stop=True)
            return sa, sb


        repl = None
        pending = None         # (blk, prods, xsm_b, p0) awaiting gtd burst

        def emit_gtd(pblk, pprods, pxsm_b, pp0):
            # gtd burst: G accumulated transposed, [c, (node, kap)]. One
            # sequential accumulation group per node region (an open matmul
            # group owns its whole PSUM bank, so groups must not interleave).
            # Emitted one block late so every product is long complete and
            # the PE never stalls mid-burst.
            gtd = pp_g.tile([C, NNOD * 64], F32, tag="g")
            for bb in range(NNOD):
                reg = gtd[:, bb * 64:(bb + 1) * 64]
                cs = slice(bb * C, (bb + 1) * C)
                nc.tensor.matmul(reg, pxsm_b[:, cs], ux3[pp0:pp0 + 16, 0:64],
                                 start=True, stop=False)
                for u in range(NUNIT):
                    nc.tensor.matmul(reg, pprods[u][:U_BLK[u + 1], cs],
                                     ua[:U_BLK[u + 1], (u + 1) * 64:(u + 2) * 64],
                                     start=False, stop=u == NUNIT - 1)
            deferred.append((pblk, gtd))

        for blk in range(NBLK):
            Lb = blk // LBLK
            p0 = 32 * Lb
            csl = slice((blk % LBLK) * NB, (blk % LBLK + 1) * NB)
            xsm_b = xsm[p0:p0 + 16, csl]

            prods = []
            if repl is None:
                repl = emit_repl(0, p0, xsm_b, prods)

            # previous block's gtd burst: fills the PE idle window while it
            # waits for this block's u0 product (which gates every t-repl)
            if pending is not None:
                emit_gtd(*pending)
                pending = None

            # ---- product units ----
            for u in range(NUNIT):
                sa, sb = repl
                rows = URON[u]
                st = spool.tile([128, NB], F32, tag="st")
                nc.scalar.copy(st[:rows], sb[:rows])
                pr = fpool.tile([128, NB], SDT, tag=f"u{u}")
                # u1's replication reads the u0 product, so it can only be
                # emitted after u0's multiply; all other units pre-emit
                # their successor's replication to keep the PE ahead
                if u > 0 and u + 1 < NUNIT:
                    repl = emit_repl(u + 1, p0, xsm_b, prods)
                elif u == NUNIT - 1 and blk + 1 < NBLK:
                    Lb1 = (blk + 1) // LBLK
                    q0 = 32 * Lb1
                    csl1 = slice(((blk + 1) % LBLK) * NB,
                                 ((blk + 1) % LBLK + 1) * NB)
                    repl = emit_repl(0, q0, xsm[q0:q0 + 16, csl1], [])
                nc.vector.tensor_mul(pr[:rows], sa[:rows], st[:rows])
                prods.append(pr)
                if u == 0:
                    repl = emit_repl(1, p0, xsm_b, prods)
                if u == 3 and deferred:
                    emit_tail(*deferred.pop(0))

            pending = (blk, prods, xsm_b, p0)

        if deferred:
            emit_tail(*deferred.pop(0))
        emit_gtd(*pending)
        while deferred:
            emit_tail(*deferred.pop(0))

        _tail(nc, tc, fpool, psa, out1, l0, l1, sct, ident, OUT, F32)
    nc.compile()
    return nc


def _tail(nc, tc, fpool, psa, out1, l0, l1, sct, ident, OUT, F32):
        import concourse.mybir as mybir
        # lin matmuls run in bf16 (l0/l1 laundered to bf16; out1 recast)
        o1b = fpool.tile([C, BLOC * 4], mybir.dt.bfloat16, tag="o1b")
        nc.vector.tensor_copy(o1b[:], out1[:])
        o1v = o1b[:].rearrange("c (b l) -> c b l", l=4)
        lo_ps = psa.tile([128, NB], F32, tag="sa")
        nc.tensor.matmul(lo_ps[:C, :BLOC], l0[:], o1v[:, :, 0], start=True, stop=True)
        l1_ps = psa.tile([128, NB], F32, tag="sa")
        nc.tensor.matmul(l1_ps[:C, :BLOC * 3].rearrange("f (b d) -> f b d", d=3),
                         l1[:], o1v[:, :, 1:4], start=True, stop=True)
        lo_sb = fpool.tile([C, BLOC], F32, tag="lo_sb")
        nc.vector.tensor_copy(lo_sb[:], lo_ps[:C, :BLOC])
        l1_sb = fpool.tile([C, BLOC * 3], F32, tag="l1_sb")
        nc.vector.tensor_copy(l1_sb[:], l1_ps[:C, :BLOC * 3])
        outt = fpool.tile([BLOC, 512], F32, tag="outt")
        tps = psa.tile([128, NB], F32, tag="sa")
        nc.tensor.transpose(tps[:BLOC, :C], lo_sb[:], ident[:])
        nc.vector.tensor_add(outt[:, 0:128], tps[:BLOC, :C], sct[:, 0:128])
        l1v = l1_sb[:].rearrange("f (b d) -> f b d", d=3)
        o_v = outt[:, 128:].rearrange("b (f d) -> b f d", d=3)
        s_v = sct[:, 128:].rearrange("b (f d) -> b f d", d=3)
        for ddi in range(3):
            tpd = psa.tile([128, NB], F32, tag="sa")
            nc.tensor.transpose(tpd[:BLOC, :C], l1v[:, :, ddi], ident[:])
            nc.vector.tensor_add(o_v[:, :, ddi], tpd[:BLOC, :C], s_v[:, :, ddi])
        nc.sync.dma_start(OUT[:], outt[:])


_PROG = {}


def kernel(**inputs):
    import concourse.bass_utils as bass_utils
    import ml_dtypes
    np_mmdt = ml_dtypes.bfloat16
    consts = _build_consts(inputs, np_mmdt)

    nf = np.asarray(inputs["node_feats"], np.float32)
    attrs = np.asarray(inputs["node_attrs"], np.float32)
    sc = np.asarray(inputs["sc"], np.float32)

    if "prog" not in _PROG:
        _PROG["prog"] = build_program()
    nc = _PROG["prog"]

    in_maps = []
    for r in range(NCORES):
        b0 = r * BLOC
        xt = nf[b0:b0 + BLOC].transpose(2, 0, 1).reshape(16, NLOC)
        # 3-lane pack: lane Lb at partition base 32*Lb holds column blocks
        # [Lb*LBLK, (Lb+1)*LBLK)
        x3 = np.zeros((80, LANEW), np.float32)
        for blk in range(NBLK):
            Lb, cb = blk // LBLK, blk % LBLK
            x3[32 * Lb:32 * Lb + 16, cb * NB:(cb + 1) * NB] = xt[:, blk * NB:(blk + 1) * NB]
        m = {"X_Tm": x3.astype(np_mmdt),
             "attrsT": np.ascontiguousarray(attrs[b0:b0 + BLOC].T),
             "sc": np.ascontiguousarray(sc[b0:b0 + BLOC])}
        m.update(consts)
        in_maps.append(m)

    res = bass_utils.run_bass_kernel_spmd(
        nc, in_maps, list(range(NCORES)),
        trace=os.environ.get("KTRACE", "0") == "1")
    global LAST_EXEC_NS
    LAST_EXEC_NS = getattr(res, "exec_time_ns", None)
    outs = [np.asarray(res.results[r]["OUT"]) for r in range(NCORES)]
    return np.concatenate(outs, axis=0).astype(np.float32)


LAST_EXEC_NS = None
